# revision 35
# baseline (speedup 1.0000x reference)
"""Trainium2 Bass kernel v6 for 2-layer GATv2 (nn_GATModel): 8-core SPMD.

v6 changes vs v5:
- conv1 source features are HOST-gathered (pure data movement): xgT[b] holds
  x[src]^T per edge slot, so xl[src] is computed per edge-tile on the PE
  (z = xgT@Wl1 + ohT@xr1). No xl1 table, no phase A, no conv1 device gathers.
- Values ride as z*e4 (not xl*e4); corrected per dst: out = pout/denom - xr1.
- One-hots (ohT dst->edge, ohB edge->dst) precomputed on host, uploaded bf16.
- Edge tiles processed in chunks of 4 with wide fused ops (prelu/mult/reduce/
  exp/vmult) to amortize fixed instruction overheads.
- BN scales folded into Wl2/Wr2/Wo on host (s>0); skip path collapsed to a
  per-node scalar ysk = x@(Ws@Wo)+bs@Wo computed in phase B.
- conv2 keeps per-tile indirect gathers (only mechanism available).
- Output y accumulated in SBUF, transposed once, single DMA.
"""

import math
import os
import sys

import numpy as np

sys.path.insert(0, os.path.dirname(os.path.abspath(__file__)))

import concourse.bass as bass  # noqa: E402
import concourse.mybir as mybir  # noqa: E402
import concourse.tile as tile  # noqa: E402
from concourse.bass import IndirectOffsetOnAxis  # noqa: E402
from concourse.bass_utils import run_bass_kernel_spmd  # noqa: E402

# ---- inlined environment workarounds (must be self-contained) ----
import concourse.tile as _ctile
from concourse import mybir as _mybir
from concourse.vector_clock import ScopedClock as _ScopedClock


def _drain_and_barrier_split(self, tick_clock, wait_clock):
    nc = self.nc
    carrier = nc.sync.nop(nofuse=True, hint="tile_exit_waits")
    wait_clock.add_sem_waits(carrier.ins, _ScopedClock({None: tick_clock.global_clock}))
    waits = list(carrier.ins.sync_info.on_wait)
    carrier.ins.sync_info.on_wait = waits[:1]
    for i in range(1, len(waits)):
        extra = nc.sync.nop(nofuse=True, hint="tile_exit_waits")
        if extra.ins.sync_info is None:
            extra.ins.sync_info = _mybir.SyncInfo(on_wait=waits[i : i + 1], on_update=[])
        else:
            extra.ins.sync_info.on_wait = waits[i : i + 1]
    nc.sync.drain()
    nc.all_engine_barrier()
    popped = nc._tile_sem_poison_stack.pop()
    assert popped is self._sem_poison
    nc.clear_and_free_semaphores(list(self.sems.allocated().values()))
    nc.all_engine_barrier()


def _install_ntff_hook():
    import sys as _s, types as _t
    import antenv
    if "antenv.axon_hooks" in _s.modules:
        return
    mod = _t.ModuleType("antenv.axon_hooks")
    _hook = [None]
    mod.set_axon_ntff_profile_hook = lambda h: _hook.__setitem__(0, h)
    mod.get_axon_ntff_profile_hook = lambda: _hook[0]
    _s.modules["antenv.axon_hooks"] = mod
    antenv.axon_hooks = mod
    try:
        from trn_agent_boot.trn_boot import _ntff_profile_via_ctypes
        mod.set_axon_ntff_profile_hook(_ntff_profile_via_ctypes("/opt/axon/libaxon_pjrt.so"))
        from concourse import bass_utils as _bu
        _bu.upload_artifacts = lambda tmpdir: f"file://{tmpdir}"
    except Exception:
        pass


def _install_patches():
    _ctile.TileContext._drain_and_barrier = _drain_and_barrier_split
    import sys as _s
    _s.path.insert(0, "/root/.axon_site")
    try:
        _install_ntff_hook()
    except Exception as e:
        print("ntff hook install failed:", e)


_install_patches()
# ---- end inlined workarounds ----


F32 = mybir.dt.float32
BF16 = mybir.dt.bfloat16
I32 = mybir.dt.int32
P = 128
NC = 8
HEADS = 4
HID = 64
D1 = HEADS * HID  # 256
IN_DIM = 160
NEG = 0.2
CH = 4
AX = mybir.AxisListType.X
AF = mybir.ActivationFunctionType
OP = mybir.AluOpType


def _split_multi_waits(nc):
    """This walrus build allows only ONE sem-wait per instruction: hoist
    extra waits onto nofuse NoOps inserted just before, on the same engine."""
    n = 0
    for fn in nc.m.functions:
        for blk in fn.blocks:
            todo = []
            for idx, inst in enumerate(blk.instructions):
                si = inst.sync_info
                if si is not None and len(si.on_wait) > 1:
                    todo.append((idx, inst))
            for idx, inst in reversed(todo):
                waits = list(inst.sync_info.on_wait)
                inst.sync_info.on_wait = waits[-1:]
                for w in waits[:-1]:
                    nop = mybir.InstNoOp(name=f"I-wsplit-{n}", ins=[], outs=[])
                    n += 1
                    nop.engine = inst.engine
                    nop.bass_nofuse = True
                    nop.sync_info = mybir.SyncInfo(on_wait=[w], on_update=[])
                    nc.register_instruction(nop)
                    blk.instructions.insert(idx, nop)
    return n


def _build(NB, TPB, KAu):
    """One SPMD program. NB blocks/core, TPB edge tiles/block (multiple of CH).
    KAu = (KA1, KA2): per-block counts of leading conv2 tiles whose sources
    all lie in AllGather thirds 1 / 1+2 (uniform across cores); those are
    pre-gathered during conv1 and staged via DRAM."""
    assert TPB % CH == 0
    KA1, KA2 = KAu
    assert len(KA1) == len(KA2) == NB and NB % 3 == 0
    NCH = TPB // CH
    nc = bass.Bass()
    dp = nc.declare_dram_parameter
    NLOC = NB * P
    TE = TPB * P  # edge slots per block

    xTl = dp("xTl", [IN_DIM, NLOC], BF16, isOutput=False)
    xgT = dp("xgT", [NB, IN_DIM, TE], BF16, isOutput=False)
    ohTd = dp("ohTd", [NB, P, TE], BF16, isOutput=False)
    ohBd = dp("ohBd", [NB, P, TE], BF16, isOutput=False)
    src2 = dp("src2", [NB, P, TPB], I32, isOutput=False)
    Wl1 = dp("Wl1", [IN_DIM, D1], BF16, isOutput=False)
    Wr1x = dp("Wr1x", [IN_DIM, D1 + 1], BF16, isOutput=False)  # [Wr1 | Ws@Wo]
    Wl2 = dp("Wl2", [D1, HID], BF16, isOutput=False)   # s1-folded
    Wr2 = dp("Wr2", [D1, HID], BF16, isOutput=False)   # s1-folded
    ident = dp("ident", [P, P], BF16, isOutput=False)
    identf = dp("identf", [P, P], F32, isOutput=False)
    b1rb = dp("b1rb", [P, D1], F32, isOutput=False)
    at1C = dp("at1C", [P, CH * D1], BF16, isOutput=False)
    cadd1 = dp("cadd1", [P, D1], F32, isOutput=False)
    b2rb = dp("b2rb", [P, HID], F32, isOutput=False)
    at2C = dp("at2C", [P, CH * HID], BF16, isOutput=False)
    cadd2 = dp("cadd2", [P, HID], F32, isOutput=False)
    wob2 = dp("wob2", [P, HID], F32, isOutput=False)   # Wo*s2 replicated
    scal = dp("scal", [P, 2], F32, isOutput=False)     # [bsWo, bo]
    y = dp("y", [NLOC, 1], F32, isOutput=True)

    xl2l = nc.dram_tensor("xl2l", [NLOC, HID], BF16)
    xl2ag2 = nc.dram_tensor("xl2ag2", [NC * NLOC, HID], BF16, addr_space="Shared")
    geT = nc.dram_tensor("geT", [NB, P, TPB * HID], BF16)

    with tile.TileContext(nc) as tc:
        with (
            tc.tile_pool(name="consts", bufs=1) as cp,
            tc.tile_pool(name="resid", bufs=1) as rp,
            tc.tile_pool(name="blk", bufs=3) as bp,
            tc.tile_pool(name="din", bufs=2) as dip,
            tc.tile_pool(name="mid", bufs=5) as mp,
            tc.tile_pool(name="gath", bufs=12) as gp,
            tc.tile_pool(name="wav", bufs=8) as wp,
            tc.tile_pool(name="small", bufs=6) as sp,
            tc.tile_pool(name="ps_z", bufs=2, space="PSUM") as ps_z,
            tc.tile_pool(name="ps_o", bufs=2, space="PSUM") as ps_o,
            tc.tile_pool(name="ps_t", bufs=1, space="PSUM") as ps_t,
        ):
            # resident constants
            idt = cp.tile([P, P], BF16)
            nc.sync.dma_start(out=idt[:], in_=ident[:])
            idtf = cp.tile([P, P], F32)
            nc.sync.dma_start(out=idtf[:], in_=identf[:])
            wl1a = cp.tile([P, D1], BF16)
            nc.sync.dma_start(out=wl1a[:], in_=Wl1[0:P, :])
            wl1b = cp.tile([IN_DIM - P, D1], BF16)
            nc.sync.dma_start(out=wl1b[:], in_=Wl1[P:IN_DIM, :])
            wr1xa = cp.tile([P, D1 + 1], BF16)
            nc.sync.dma_start(out=wr1xa[:], in_=Wr1x[0:P, :])
            wr1xb = cp.tile([IN_DIM - P, D1 + 1], BF16)
            nc.sync.dma_start(out=wr1xb[:], in_=Wr1x[P:IN_DIM, :])
            wl2a = cp.tile([P, HID], BF16)
            nc.sync.dma_start(out=wl2a[:], in_=Wl2[0:P, :])
            wl2b = cp.tile([P, HID], BF16)
            nc.sync.dma_start(out=wl2b[:], in_=Wl2[P:D1, :])
            wr2a = cp.tile([P, HID], BF16)
            nc.sync.dma_start(out=wr2a[:], in_=Wr2[0:P, :])
            wr2b = cp.tile([P, HID], BF16)
            nc.sync.dma_start(out=wr2b[:], in_=Wr2[P:D1, :])
            b1r = cp.tile([P, D1], F32)
            nc.sync.dma_start(out=b1r[:], in_=b1rb[:])
            at1 = cp.tile([P, CH * D1], BF16)
            nc.sync.dma_start(out=at1[:], in_=at1C[:])
            c1a = cp.tile([P, D1], F32)
            nc.sync.dma_start(out=c1a[:], in_=cadd1[:])
            b2r = cp.tile([P, HID], F32)
            nc.sync.dma_start(out=b2r[:], in_=b2rb[:])
            at2 = cp.tile([P, CH * HID], BF16)
            nc.sync.dma_start(out=at2[:], in_=at2C[:])
            c2a = cp.tile([P, HID], F32)
            nc.sync.dma_start(out=c2a[:], in_=cadd2[:])
            wo2 = cp.tile([P, HID], F32)
            nc.sync.dma_start(out=wo2[:], in_=wob2[:])
            sc2 = cp.tile([P, 2], F32)
            nc.sync.dma_start(out=sc2[:], in_=scal[:])

            # resident state
            xr1R = rp.tile([P, NB * D1], BF16)
            xr2R = rp.tile([P, NB * HID], BF16)
            yskR = rp.tile([P, NB], F32)
            hvR = rp.tile([P, NB * D1], BF16)
            h2R = rp.tile([P, NB * HID], BF16)

            # ---- phase B: local xr1 (+b1r), skip scalar ----
            for m in range(NB):
                xa = dip.tile([P, P], BF16, tag="xa")
                nc.sync.dma_start(out=xa[:], in_=xTl[0:P, m * P : (m + 1) * P])
                xb = dip.tile([IN_DIM - P, P], BF16, tag="xb")
                nc.sync.dma_start(out=xb[:], in_=xTl[P:IN_DIM, m * P : (m + 1) * P])
                pz = ps_o.tile([P, D1 + 4], F32, tag="pout")
                nc.tensor.matmul(out=pz[:, 0 : D1 + 1], lhsT=xa[:], rhs=wr1xa[:],
                                 start=True, stop=False)
                nc.tensor.matmul(out=pz[:, 0 : D1 + 1], lhsT=xb[:], rhs=wr1xb[:],
                                 start=False, stop=True)
                nc.vector.tensor_add(out=xr1R[:, m * D1 : (m + 1) * D1],
                                     in0=pz[:, 0:D1], in1=b1r[:])
                nc.vector.tensor_scalar(
                    out=yskR[:, m : m + 1], in0=pz[:, D1 : D1 + 1],
                    scalar1=sc2[:, 0:1], scalar2=None, op0=OP.add)

            # ---- conv1 edge stage (per-block epilogue incl. BN1+relu+W2) ----
            TH = NB // 3
            SEGR = TH * P

            def conv1_block(b):
                oht = bp.tile([P, TE], BF16, tag="oht")
                nc.sync.dma_start(out=oht[:], in_=ohTd[b])
                ohb = bp.tile([P, TE], BF16, tag="ohb")
                nc.sync.dma_start(out=ohb[:], in_=ohBd[b])
                xga = bp.tile([P, TE], BF16, tag="xga")
                nc.sync.dma_start(out=xga[:], in_=xgT[b, 0:P, :])
                xgb = bp.tile([IN_DIM - P, TE], BF16, tag="xgb")
                nc.sync.dma_start(out=xgb[:], in_=xgT[b, P:IN_DIM, :])
                xr1b = xr1R[:, b * D1 : (b + 1) * D1]
                pout = ps_o.tile([P, D1 + 4], F32, tag="pout")
                for c in range(NCH):
                    pz = ps_z.tile([P, CH, D1], F32, tag="pz")
                    for j in range(CH):
                        t = c * CH + j
                        sl = slice(t * P, (t + 1) * P)
                        nc.tensor.matmul(out=pz[:, j, :], lhsT=xga[:, sl], rhs=wl1a[:],
                                         start=True, stop=False)
                        nc.tensor.matmul(out=pz[:, j, :], lhsT=xgb[:, sl], rhs=wl1b[:],
                                         start=False, stop=False)
                    xlc = mp.tile([P, CH, D1], BF16, tag="xlc")
                    nc.scalar.copy(out=xlc[:], in_=pz[:])
                    for j in range(CH):
                        t = c * CH + j
                        sl = slice(t * P, (t + 1) * P)
                        nc.tensor.matmul(out=pz[:, j, :], lhsT=oht[:, sl], rhs=xr1b,
                                         start=False, stop=True)
                    lr = mp.tile([P, CH * D1], BF16, tag="lr")
                    nc.scalar.activation(out=lr[:], in_=pz[:], func=AF.Prelu, alpha=NEG)
                    tt = mp.tile([P, CH * HEADS, HID], BF16, tag="tt")
                    nc.vector.tensor_mul(out=tt[:], in0=lr[:], in1=at1[:])
                    lg = sp.tile([P, CH * HEADS], F32, tag="lg")
                    nc.vector.reduce_sum(out=lg[:], in_=tt[:], axis=AX)
                    v = mp.tile([P, CH, 4 + D1], BF16, tag="v")
                    nc.scalar.activation(out=v[:, :, 0:4], in_=lg[:], func=AF.Exp)
                    nc.vector.tensor_tensor(
                        out=v[:, :, 4 : 4 + D1],
                        in0=v[:, :, 0:4].to_broadcast([P, CH, HEADS, HID]),
                        in1=xlc[:],
                        op=OP.mult,
                    )
                    for j in range(CH):
                        t = c * CH + j
                        nc.tensor.matmul(
                            out=pout[:],
                            lhsT=ohb[:, t * P : (t + 1) * P],
                            rhs=v[:, j, :],
                            start=(t == 0),
                            stop=(t == TPB - 1),
                        )
                # epilogue: h = pout/denom + c1/s1, relu; then W2 transposes/MMs
                r4 = sp.tile([P, HEADS], F32, tag="r4")
                nc.vector.reciprocal(out=r4[:], in_=pout[:, 0:4])
                hvb = hvR[:, b * D1 : (b + 1) * D1]
                nc.vector.tensor_tensor(
                    out=hvb, in0=pout[:, 4 : 4 + D1],
                    in1=r4[:].to_broadcast([P, HEADS, HID]), op=OP.mult)
                nc.vector.tensor_tensor(out=hvb, in0=hvb, in1=c1a[:], op=OP.add)
                nc.vector.tensor_scalar(
                    out=hvb, in0=hvb, scalar1=0.0, scalar2=None, op0=OP.max)
                h1 = hvR[:, b * D1 : b * D1 + P]
                h2 = hvR[:, b * D1 + P : (b + 1) * D1]
                pt1 = ps_t.tile([P, P], BF16, tag="ptb")
                nc.tensor.transpose(out=pt1[:], in_=h1, identity=idt[:])
                hT1 = mp.tile([P, P], BF16, tag="hT1")
                nc.scalar.copy(out=hT1[:], in_=pt1[:])
                pt2 = ps_t.tile([P, P], BF16, tag="ptb")
                nc.tensor.transpose(out=pt2[:], in_=h2, identity=idt[:])
                hT2 = mp.tile([P, P], BF16, tag="hT2")
                nc.scalar.copy(out=hT2[:], in_=pt2[:])
                pl2 = ps_t.tile([P, P], F32, tag="pw2")
                nc.tensor.matmul(out=pl2[:, 0:HID], lhsT=hT1[:], rhs=wl2a[:], start=True, stop=False)
                nc.tensor.matmul(out=pl2[:, 0:HID], lhsT=hT2[:], rhs=wl2b[:], start=False, stop=True)
                l2t = mp.tile([P, HID], BF16, tag="l2t")
                nc.scalar.copy(out=l2t[:], in_=pl2[:, 0:HID])
                nc.sync.dma_start(out=xl2l[b * P : (b + 1) * P, :], in_=l2t[:])
                pr2 = ps_t.tile([P, P], F32, tag="pw2")
                nc.tensor.matmul(out=pr2[:, 0:HID], lhsT=hT1[:], rhs=wr2a[:], start=True, stop=False)
                nc.tensor.matmul(out=pr2[:, 0:HID], lhsT=hT2[:], rhs=wr2b[:], start=False, stop=True)
                nc.vector.tensor_add(out=xr2R[:, b * HID : (b + 1) * HID],
                                     in0=pr2[:, 0:HID], in1=b2r[:])

            def wave_block(w, lo, hi):
                # pre-gather conv2 tiles [lo, hi) of block w into DRAM geT;
                # write back per chunk of CH tiles so each sync-queue write
                # waits on at most CH gathers
                if hi <= lo:
                    return
                idxA = bp.tile([P, TPB], I32, tag="idxA")
                nc.sync.dma_start(out=idxA[:], in_=src2[w])
                t = lo
                while t < hi:
                    n = min(CH, hi - t)
                    ga = wp.tile([P, CH, HID], BF16, tag="ga")
                    for j in range(n):
                        nc.gpsimd.indirect_dma_start(
                            out=ga[:, j, :],
                            out_offset=None,
                            in_=xl2ag2[:],
                            in_offset=IndirectOffsetOnAxis(
                                ap=idxA[:, t + j : t + j + 1], axis=0),
                        )
                    nc.sync.dma_start(out=geT[w][:, t * HID : (t + n) * HID],
                                      in_=ga[:, 0:n, :])
                    t += n

            def emit_ag(k):
                nc.gpsimd.collective_compute(
                    "AllGather", OP.bypass, replica_groups=[list(range(NC))],
                    ins=[xl2l[k * SEGR : (k + 1) * SEGR, :]],
                    outs=[xl2ag2[k * NC * SEGR : (k + 1) * NC * SEGR, :]])

            for b in range(TH):
                conv1_block(b)
            emit_ag(0)
            wptr = 0
            for b in range(TH, 2 * TH):
                conv1_block(b)
                for _ in range(3):
                    if wptr < NB:
                        wave_block(wptr, 0, KA1[wptr])
                        wptr += 1
            while wptr < NB:
                wave_block(wptr, 0, KA1[wptr])
                wptr += 1
            emit_ag(1)
            wptr = 0
            for b in range(2 * TH, NB):
                conv1_block(b)
                for _ in range(3):
                    if wptr < NB:
                        wave_block(wptr, KA1[wptr], KA2[wptr])
                        wptr += 1
            while wptr < NB:
                wave_block(wptr, KA1[wptr], KA2[wptr])
                wptr += 1
            emit_ag(2)

            # ---- conv2 edge stage ----
            for b in range(NB):
                oht = bp.tile([P, TE], BF16, tag="oht")
                nc.sync.dma_start(out=oht[:], in_=ohTd[b])
                ohb = bp.tile([P, TE], BF16, tag="ohb")
                nc.sync.dma_start(out=ohb[:], in_=ohBd[b])
                idx2 = bp.tile([P, TPB], I32, tag="idx2")
                nc.sync.dma_start(out=idx2[:], in_=src2[b])
                xr2b = xr2R[:, b * HID : (b + 1) * HID]
                pout = ps_o.tile([P, D1 + 4], F32, tag="pout")
                for c in range(NCH):
                    t0 = c * CH
                    nA = min(max(KA2[b] - t0, 0), CH)
                    xg2 = gp.tile([P, CH, HID], BF16, tag="xg2")
                    if nA > 0:
                        nc.sync.dma_start(
                            out=xg2[:, 0:nA, :],
                            in_=geT[b][:, t0 * HID : (t0 + nA) * HID])
                    for j in range(nA, CH):
                        t = t0 + j
                        nc.gpsimd.indirect_dma_start(
                            out=xg2[:, j, :],
                            out_offset=None,
                            in_=xl2ag2[:],
                            in_offset=IndirectOffsetOnAxis(ap=idx2[:, t : t + 1], axis=0),
                        )
                    pz = ps_z.tile([P, CH, D1], F32, tag="pz")
                    for j in range(CH):
                        nc.tensor.matmul(out=pz[:, j, 0:HID], lhsT=idt[:], rhs=xg2[:, j, :],
                                         start=True, stop=False)
                    for j in range(CH):
                        t = c * CH + j
                        nc.tensor.matmul(out=pz[:, j, 0:HID], lhsT=oht[:, t * P : (t + 1) * P],
                                         rhs=xr2b, start=False, stop=True)
                    lr2 = mp.tile([P, CH, HID], BF16, tag="lr2")
                    nc.scalar.activation(out=lr2[:], in_=pz[:, :, 0:HID], func=AF.Prelu, alpha=NEG)
                    tt2 = mp.tile([P, CH, HID], BF16, tag="tt2")
                    nc.vector.tensor_mul(out=tt2[:], in0=lr2[:], in1=at2[:])
                    lg2 = sp.tile([P, CH], F32, tag="lg2")
                    nc.vector.reduce_sum(out=lg2[:], in_=tt2[:], axis=AX)
                    v2 = mp.tile([P, CH, 1 + HID], BF16, tag="v2")
                    nc.scalar.activation(out=v2[:, :, 0:1], in_=lg2[:], func=AF.Exp)
                    nc.vector.tensor_tensor(
                        out=v2[:, :, 1 : 1 + HID],
                        in0=v2[:, :, 0:1].to_broadcast([P, CH, 1, HID]),
                        in1=xg2[:],
                        op=OP.mult,
                    )
                    for j in range(CH):
                        t = c * CH + j
                        nc.tensor.matmul(
                            out=pout[:, 0 : 1 + HID],
                            lhsT=ohb[:, t * P : (t + 1) * P],
                            rhs=v2[:, j, :],
                            start=(t == 0),
                            stop=(t == TPB - 1),
                        )
                r1 = sp.tile([P, 1], F32, tag="r1")
                nc.vector.reciprocal(out=r1[:], in_=pout[:, 0:1])
                h2b = h2R[:, b * HID : (b + 1) * HID]
                nc.vector.tensor_tensor(
                    out=h2b, in0=pout[:, 1 : 1 + HID],
                    in1=r1[:].to_broadcast([P, 1, HID]), op=OP.mult)

            # ---- wide BN2 + relu + Wo reduce + skip + bo ----
            nc.vector.tensor_tensor(
                out=h2R[:].rearrange("p (b f) -> p f b", b=NB),
                in0=h2R[:].rearrange("p (b f) -> p f b", b=NB),
                in1=c2a[:].to_broadcast([P, HID, NB]),
                op=OP.add,
            )
            nc.vector.tensor_scalar(
                out=h2R[:], in0=h2R[:], scalar1=0.0, scalar2=None, op0=OP.max)
            hw = rp.tile([P, NB, HID], F32)
            nc.vector.tensor_tensor(
                out=hw[:].rearrange("p b f -> p f b"),
                in0=h2R[:].rearrange("p (b f) -> p f b", b=NB),
                in1=wo2[:].to_broadcast([P, HID, NB]),
                op=OP.mult,
            )
            yv = rp.tile([P, NB], F32)
            nc.vector.reduce_sum(out=yv[:], in_=hw[:], axis=AX)
            nc.vector.tensor_add(out=yv[:], in0=yv[:], in1=yskR[:])
            nc.vector.tensor_scalar(
                out=yv[:], in0=yv[:], scalar1=sc2[:, 1:2], scalar2=None, op0=OP.add)
            pty = ps_t.tile([P, P], F32, tag="pw2")
            nc.tensor.transpose(out=pty[0:NB, :], in_=yv[:], identity=idtf[:])
            yt = mp.tile([NB, P], F32, tag="yt")
            nc.scalar.copy(out=yt[:], in_=pty[0:NB, :])
            nc.sync.dma_start(
                out=y[:].rearrange("(b p) one -> b (p one)", b=NB), in_=yt[:])
    _split_multi_waits(nc)
    return nc


def _host_prep(x, edge_index, Wl1, bl1, Wr1, br1, att1, bias1, g1, b1, m1, v1,
               Wl2, bl2, Wr2, br2, att2, bias2, g2, b2, m2, v2, Ws, bs, Wo, bo,
               NB):
    """Numpy-side graph partitioning + host gather + constant folding."""
    import ml_dtypes
    bf = lambda a: np.asarray(a, np.float32).astype(ml_dtypes.bfloat16)
    x = np.asarray(x, np.float32)
    N = x.shape[0]
    src = np.concatenate([edge_index[0], np.arange(N, dtype=np.int64)]).astype(np.int64)
    dst = np.concatenate([edge_index[1], np.arange(N, dtype=np.int64)]).astype(np.int64)

    NBINS = NC * NB
    deg = np.bincount(dst, minlength=N)
    order = np.argsort(-deg, kind="stable")
    i = np.arange(N)
    r = i // NBINS
    pos = i % NBINS
    bsel = np.where(r % 2 == 0, pos, NBINS - 1 - pos)
    binof = np.empty(N, np.int64)
    slotof = np.empty(N, np.int64)
    binof[order] = bsel
    slotof[order] = r
    assert slotof.max() < P

    NLOC = NB * P
    assert NB % 3 == 0
    SEGR = (NB // 3) * P
    agof = (binof // NB) * (NB * P) + (binof % NB) * P + slotof

    ebin = binof[dst]
    # sort each bin's edges by which AllGather third the source falls in
    tsrc = ((agof[src] % NLOC) // SEGR).astype(np.int64)
    eorder = np.argsort(ebin * 4 + tsrc, kind="stable")
    counts = np.bincount(ebin, minlength=NBINS)
    TPB = int(math.ceil(counts.max() / P))
    TPB = ((TPB + CH - 1) // CH) * CH
    TE = TPB * P
    offs = np.zeros(NBINS + 1, np.int64)
    np.cumsum(counts, out=offs[1:])
    pwc = np.arange(len(src)) - offs[ebin[eorder]]

    eb = ebin[eorder]
    es = src[eorder]
    ed = dst[eorder]
    pp = (pwc % P).astype(np.int64)
    tt = (pwc // P).astype(np.int64)
    flat = tt * P + pp

    # conv2 gather rows in the split-AG address space: third k of every
    # core's rows pack at [k*NC*SEGR, (k+1)*NC*SEGR)
    r_ag = agof[es]
    r_c = r_ag // NLOC
    r_r = r_ag % NLOC
    r_t = r_r // SEGR
    row3 = r_t * (NC * SEGR) + r_c * SEGR + (r_r - r_t * SEGR)

    srcslot = np.full((NBINS, TE), -1, np.int64)
    srcslot[eb, flat] = es
    dstslot = np.full((NBINS, TE), -1, np.int64)
    dstslot[eb, flat] = slotof[ed]
    src2_arr = np.zeros((NBINS, P, TPB), np.int32)
    src2_arr[eb, pp, tt] = row3.astype(np.int32)

    # per-block uniform counts of leading tiles pure in thirds 1 / 1+2
    ecore = eb // NB
    c1 = np.bincount(ebin[tsrc == 0], minlength=NBINS)
    c12 = np.bincount(ebin[tsrc <= 1], minlength=NBINS)
    KA1 = (c1 // P).reshape(NC, NB).min(axis=0)
    KA2 = (c12 // P).reshape(NC, NB).min(axis=0)
    KAu = (tuple(int(min(k, TPB)) for k in KA1),
           tuple(int(min(k, TPB)) for k in KA2))

    # one-hots (built directly in bf16)
    import ml_dtypes as _mld
    cols = np.arange(TE)
    ohT = np.zeros((NBINS, P, TE), _mld.bfloat16)
    ohB = np.zeros((NBINS, P, TE), _mld.bfloat16)  # [pp, tt*P + d] edge-major
    one = _mld.bfloat16(1.0)
    for bb in range(NBINS):
        ds = dstslot[bb]
        ok = ds >= 0
        ohT[bb][ds[ok], cols[ok]] = one
        ppv = cols[ok] % P
        ttv = cols[ok] // P
        ohB[bb][ppv, ttv * P + ds[ok]] = one

    # host gather of x for conv1 sources, transposed per block (bf16 direct)
    xT = np.ascontiguousarray(x.T.astype(_mld.bfloat16))  # [160, N]
    xg_all = np.zeros((NBINS, IN_DIM, TE), _mld.bfloat16)
    valid = srcslot >= 0
    for bb in range(NBINS):
        ok = valid[bb]
        xg_all[bb][:, ok] = xT[:, srcslot[bb][ok]]

    node_of = np.full((NBINS, P), -1, np.int64)
    node_of[binof, slotof] = np.arange(N)

    # constant folding
    f32 = np.float32
    s1 = (g1 / np.sqrt(v1 + 1e-5)).astype(f32)
    c1 = ((bias1 + bl1 - m1) * s1 + b1).astype(f32)
    s2 = (g2 / np.sqrt(v2 + 1e-5)).astype(f32)
    c2 = ((bias2 + bl2 - m2) * s2 + b2).astype(f32)
    cadd1 = (c1 / s1).astype(f32)
    cadd2 = (c2 / s2).astype(f32)
    Wl2f = (np.asarray(Wl2, f32) * s1[:, None]).astype(f32)
    Wr2f = (np.asarray(Wr2, f32) * s1[:, None]).astype(f32)
    WsWo = (np.asarray(Ws, f32) @ np.asarray(Wo, f32).reshape(-1, 1)).astype(f32)  # [160,1]
    Wr1x = np.concatenate([np.asarray(Wr1, f32), WsWo], axis=1)  # [160, 257]
    bsWo = float(np.asarray(bs, f32) @ np.asarray(Wo, f32).reshape(-1))
    wob2v = (np.asarray(Wo, f32).reshape(-1) * s2).astype(f32)
    rep = lambda vv: np.tile(np.asarray(vv, f32).reshape(1, -1), (P, 1))

    common = dict(
        Wl1=bf(Wl1), Wr1x=bf(Wr1x), Wl2=bf(Wl2f), Wr2=bf(Wr2f),
        ident=bf(np.eye(P, dtype=f32)),
        identf=np.eye(P, dtype=f32),
        b1rb=rep(np.asarray(bl1, f32) + np.asarray(br1, f32)),
        at1C=bf(np.tile(rep(np.asarray(att1, f32).reshape(-1)), (1, CH))),
        cadd1=rep(cadd1),
        b2rb=rep(np.asarray(bl2, f32) + np.asarray(br2, f32)),
        at2C=bf(np.tile(rep(np.asarray(att2, f32).reshape(-1)), (1, CH))),
        cadd2=rep(cadd2),
        wob2=rep(wob2v),
        scal=np.tile(np.array([[bsWo, float(np.asarray(bo).reshape(-1)[0])]], f32), (P, 1)),
    )

    in_maps = []
    for cc in range(NC):
        bins = slice(cc * NB, (cc + 1) * NB)
        nid = node_of[bins].reshape(-1)
        xl = np.zeros((NB * P, IN_DIM), np.float32)
        ok = nid >= 0
        xl[ok] = x[nid[ok]]
        m = dict(common)
        m.update(
            xTl=bf(np.ascontiguousarray(xl.T)),
            xgT=xg_all[bins],
            ohTd=ohT[bins],
            ohBd=ohB[bins],
            src2=src2_arr[bins],
        )
        in_maps.append(m)

    meta = dict(TPB=TPB, agof=agof, N=N, KAu=KAu)
    return in_maps, meta


_PROG_CACHE = {}


def kernel(**inputs):
    NB = 54
    inp = {k: np.asarray(v) for k, v in inputs.items()}
    x = inp["x"].astype(np.float32)
    in_maps, meta = _host_prep(
        x, inp["edge_index"], inp["Wl1"], inp["bl1"], inp["Wr1"], inp["br1"],
        inp["att1"], inp["bias1"], inp["g1"], inp["b1"], inp["m1"], inp["v1"],
        inp["Wl2"], inp["bl2"], inp["Wr2"], inp["br2"], inp["att2"], inp["bias2"],
        inp["g2"], inp["b2"], inp["m2"], inp["v2"], inp["Ws"], inp["bs"],
        inp["Wo"], inp["bo"], NB,
    )
    key = (NB, meta["TPB"], meta["KAu"])
    if key not in _PROG_CACHE:
        _PROG_CACHE[key] = _build(*key)
    nc = _PROG_CACHE[key]
    res = run_bass_kernel_spmd(nc, in_maps, list(range(NC)))
    ylin = np.concatenate([res.results[c]["y"].reshape(-1) for c in range(NC)])
    return ylin[meta["agof"]].astype(np.float32)
